# revision 35
# baseline (speedup 1.0000x reference)
"""Trainium2 Bass kernel for nn_MultiHeadQuantileNBEATS.

Reference computation (per batch row b):
  feats = x @ W_bb + b_bb                                   [D]
  h1[q] = relu(feats @ W1[q] + b1[q])                       [QF, H1]
  h2[q] = relu(h1[q] @ W2[q] + b2[q])                       [QF, H2]
  o3[q] = h2[q] @ W3[q] + b3[q]                             [QF, HOR]
  sq    = sort(o3 over q)  (per (b, hor))                   [HOR, QF]
  out[b, h, t] = sort_t(interp(sq[b, h, :], q[b, t]))       [HOR, QT]

Device algorithm notes:
  * Pure data parallel over 8 cores (batch sharded, weights replicated).
  * Head math is feature major ([feature, batch]); x arrives pre-transposed
    from the host so no on-chip input transpose is needed.
  * All matmuls are true fp32: the final interp is a convex combination
    whose result can be ~1e-3 while operands are ~0.5, so any reduced
    precision on the value path (fp32r/tf32 matmuls, low-precision storage
    of sorted values or coefficients) is amplified ~500x by the
    scale-relative error metric and fails the 2e-2 gate.  Only the final
    output store is bf16 (rounding the result itself is a plain <=0.4%
    relative error).
  * The final sort over QT is eliminated: the interpolant is monotone in the
    query level, so using host-sorted q yields already-sorted outputs.
  * Interpolation coefficients a_i(q[b,t]) are precomputed on the host in the
    transposed [(s,i), t] group layout, so the device only transposes the
    sorted head outputs (PE is_transpose path, 4 groups batched per PSUM
    bank with a single evacuation).
  * The block-diagonal coefficient matrix A is built 4 groups per DVE/Pool op
    (broadcast * 0/1 mask).
  * PSUM evacuations are spread across Act (bias/relu fused), DVE and Pool
    (gpsimd); result tiles round-robin across all three.
  * Per-core output is written feature-major [HOR, B_core, QT]; the host
    transposes to [B, HOR, QT] when gathering.
"""

import dataclasses
from contextlib import ExitStack

import numpy as np

import concourse.bass as bass
import concourse.mybir as mybir
import concourse.tile as tile
from concourse import bass_utils
from concourse.bass import ts
from concourse.masks import make_identity

F32 = mybir.dt.float32
BF16 = mybir.dt.bfloat16

B, T, D = 8192, 512, 512
H1, H2, HOR = 256, 128, 96
QF, QT = 7, 32
NCORES = 8
BC = B // NCORES  # batch per core
SUB = 512         # samples per super-tile
GRP = 16          # samples per interp group
GB = 4            # interp groups batched per PSUM bank / A tile
QUANTILE_LEVELS = np.array(
    [0.025, 0.1, 0.25, 0.5, 0.75, 0.9, 0.975], dtype=np.float32
)

# optimal 16-CE sorting network for 7 elements (ascending), disjoint layers
SORT7_LAYERS = [
    [(1, 2), (3, 4), (5, 6)],
    [(0, 2), (3, 5), (4, 6)],
    [(0, 1), (4, 5), (2, 6)],
    [(0, 4), (1, 5)],
    [(0, 3), (2, 5)],
    [(1, 3), (2, 4)],
    [(2, 3)],
]


def _view(ap, free_dims, extra_offset):
    """Rebuild an AP keeping its partition dim, with custom free-dim lattice."""
    dims = [tuple(ap.ap[0])] + [tuple(d) for d in free_dims]
    return dataclasses.replace(ap, ap=tuple(dims), offset=ap.offset + extra_offset)


# ---------------------------------------------------------------------------
# host-side constants / input prep
# ---------------------------------------------------------------------------

def _host_constants(b_eff, b2, b3):
    # bias_all [128, 32]: packed per-partition bias columns
    bias = np.zeros((128, 32), dtype=np.float32)
    for qh in range(QF):
        for mc in range(H1 // 128):
            bias[:, 4 + 2 * qh + mc] = b_eff[qh, 128 * mc : 128 * (mc + 1)]
        bias[:, 18 + qh] = b2[qh]
        bias[:96, 25 + qh] = b3[qh]
    # M112 [112, 512]: block-diagonal 0/1 mask over (sample, coeff) x (sample, t)
    m112 = np.zeros((112, 512), dtype=np.float32)
    for i in range(QF):
        for s in range(GRP):
            m112[GRP * i + s, QT * s : QT * s + QT] = 1.0
    return bias, m112


def _host_coeffs(q_sorted):
    """Interp coefficients for sorted q, in transposed group layout.

    Returns aT packed as [7*GRP, (B//GRP)*QT] fp32 where
    aT[7*sl + i, QT*g + t] = a_i(q_sorted[GRP*g + sl, t]).
    """
    ql = QUANTILE_LEVELS
    v = q_sorted  # [B, QT] fp32
    hi = np.clip(np.searchsorted(ql, v, side="left"), 1, QF - 1)
    lo = hi - 1
    w = ((v - ql[lo]) / (ql[hi] - ql[lo] + np.float32(1e-8))).astype(np.float32)
    a = np.zeros((v.shape[0], QT, QF), dtype=np.float32)
    np.put_along_axis(a, lo[:, :, None], (1.0 - w)[:, :, None], axis=2)
    np.put_along_axis(a, hi[:, :, None], w[:, :, None], axis=2)
    low_mask = v <= ql[0]
    high_mask = v >= ql[-1]
    a[low_mask] = 0.0
    a[high_mask] = 0.0
    a[..., 0] += low_mask.astype(np.float32)
    a[..., QF - 1] += high_mask.astype(np.float32)
    ngrp = v.shape[0] // GRP
    # [B, QT, QF] -> [ngrp, GRP(sl), QT, QF] -> [QF, GRP, ngrp, QT]
    # partition p = GRP*i + sl  (i-major keeps device-side writes contiguous)
    aT = a.reshape(ngrp, GRP, QT, QF).transpose(3, 1, 0, 2)
    return np.ascontiguousarray(aT.reshape(QF * GRP, ngrp * QT))


# ---------------------------------------------------------------------------
# device kernel
# ---------------------------------------------------------------------------

HEAD_ORDER = [1, 2, 3, 4, 5, 6, 0]
HEAD_PAIRS = [(1, 2), (3, 4), (5, 6), (0, None)]


def _emit(ctx: ExitStack, tc: tile.TileContext, ins, outs, bc=BC):
    nc = tc.nc
    xT_d, aT_d, w1_d, w2_d, w3_d, bias_d, m112_d = ins
    (r_d,) = outs
    n_sub = bc // SUB
    ngrp = SUB // GRP

    cpool = ctx.enter_context(tc.tile_pool(name="cpool", bufs=1))
    wpool = ctx.enter_context(tc.tile_pool(name="wpool", bufs=1))
    atpool = ctx.enter_context(tc.tile_pool(name="atpool", bufs=2))
    xTpool = ctx.enter_context(tc.tile_pool(name="xTpool", bufs=2))
    h1pool = ctx.enter_context(tc.tile_pool(name="h1pool", bufs=2))
    h2pool = ctx.enter_context(tc.tile_pool(name="h2pool", bufs=2))
    o3pool = ctx.enter_context(tc.tile_pool(name="o3pool", bufs=12))
    sqgpool = ctx.enter_context(tc.tile_pool(name="sqgpool", bufs=2))
    sqTpool = ctx.enter_context(tc.tile_pool(name="sqTpool", bufs=3))
    apool = ctx.enter_context(tc.tile_pool(name="apool", bufs=2))
    rpool = ctx.enter_context(tc.tile_pool(name="rpool", bufs=5))
    tpsum = ctx.enter_context(tc.tile_pool(name="tpsum", bufs=2, space="PSUM"))
    hpsum = ctx.enter_context(tc.tile_pool(name="hpsum", bufs=3, space="PSUM"))
    rpsum = ctx.enter_context(tc.tile_pool(name="rpsum", bufs=3, space="PSUM"))

    # --- constants ---
    ident = cpool.tile([128, 128], F32)
    make_identity(nc, ident[:])
    bias_sb = cpool.tile([128, 32], F32)
    nc.sync.dma_start(bias_sb[:], bias_d)
    m112 = cpool.tile([112, 512], F32)
    nc.sync.dma_start(m112[:], m112_d)

    # ---- super-tile 0 input loads FIRST so the PE can start as soon as the
    # first W_eff chunks land (weights queue behind on the same DMA queue) ----
    xT_st = []
    aT_st = []
    for st in range(n_sub):
        xT_st.append([None] * 4)
        aT_st.append(None)
    for tci in range(4):
        xt = xTpool.tile([128, SUB], F32, name=f"xT0_{tci}", tag=f"xT{tci}")
        nc.scalar.dma_start(xt[:], xT_d[ts(tci, 128), ts(0, SUB)])
        xT_st[0][tci] = xt

    # --- weights (persist across super-tiles); W_eff = W_bb @ W1 host-fused.
    # DMA'd per head in consumption order so the per-head pipelines never
    # wait on a weight that is queued behind unrelated ones ---
    w1_sb = [None] * QF
    w2_sb = [None] * QF
    w3_sb = [None] * QF
    for li_, qh in enumerate(HEAD_ORDER):
        row = []
        for dc in range(D // 128):
            w = wpool.tile([128, H1], F32, name=f"w1_{qh}_{dc}")
            nc.sync.dma_start(w[:], w1_d[qh, ts(dc, 128), :])
            row.append(w)
        w1_sb[qh] = row
        row = []
        for mc in range(H1 // 128):
            w = wpool.tile([128, H2], F32, name=f"w2_{qh}_{mc}")
            nc.sync.dma_start(w[:], w2_d[qh, ts(mc, 128), :])
            row.append(w)
        w2_sb[qh] = row
        w = wpool.tile([128, HOR], F32, name=f"w3_{qh}")
        nc.sync.dma_start(w[:], w3_d[qh])
        w3_sb[qh] = w
        if li_ == 1:
            aT0 = atpool.tile([112, ngrp * QT], F32, name="aT0", tag="aT")
            nc.scalar.dma_start(aT0[:], aT_d[:, ts(0, ngrp * QT)])
            aT_st[0] = aT0

    # =====================================================================
    # heads + sort + interpolation.  Work is emitted per super-tile, with the
    # previous super-tile's interp batches WOVEN between the next one's head
    # pairs: engines execute their streams in order, so emission order is the
    # schedule, and interleaving keeps Act/DVE from building a backlog that
    # stalls the PE's W2/W3 dependency chains.
    # =====================================================================
    HGRP = ngrp // 2   # groups per half
    HSUB = SUB // 2    # samples per half
    last_touch = {}
    for li, layer in enumerate(SORT7_LAYERS):
        for (a, b) in layer:
            last_touch[a] = (li, a, b)
            last_touch[b] = (li, a, b)

    def make_st(st):
        """Build the emission closures for one super-tile."""
        S = {}

        def emit_loads():
            if xT_st[st][0] is None:
                for tci in range(4):
                    xt = xTpool.tile([128, SUB], F32, name=f"xT{st}_{tci}",
                                     tag=f"xT{tci}")
                    nc.sync.dma_start(xt[:], xT_d[ts(tci, 128), ts(st, SUB)])
                    xT_st[st][tci] = xt
                a_t = atpool.tile([112, ngrp * QT], F32, name=f"aT{st}",
                                  tag="aT")
                nc.sync.dma_start(a_t[:], aT_d[:, ts(st, ngrp * QT)])
                aT_st[st] = a_t
            S["xT"] = xT_st[st]
            S["aT"] = aT_st[st]
            S["o3"] = [None] * QF
            S["cur"] = [{}, {}]
            S["pending"] = [(li, a, b) for li, layer in enumerate(SORT7_LAYERS)
                            for (a, b) in layer]
            S["sqgs"] = [
                sqgpool.tile([HOR, HGRP * 112], F32, name=f"sqg{st}_{hf}",
                             tag=f"sqg{hf}")
                for hf in range(2)
            ]
            S["A"] = {}

        def build_A(bi):
            A = apool.tile([112, GB * 512], F32, name=f"A{st}_{bi}", tag="A",
                           bufs=6)
            av = S["aT"][:, GB * QT * bi : GB * QT * (bi + 1)].rearrange(
                "p (j t) -> p j t", j=GB
            ).unsqueeze(2).broadcast_to((112, GB, GRP, QT))
            mv = m112[:].rearrange("p (s t) -> p s t", s=GRP).unsqueeze(
                1
            ).broadcast_to((112, GB, GRP, QT))
            Av = A[:].rearrange("p (j s t) -> p j s t", j=GB, s=GRP)
            nc.vector.tensor_tensor(Av, av, mv, op=mybir.AluOpType.mult)
            return A

        def emit_ce(hf, li, a, b):
            c0 = HSUB * hf
            sqg = S["sqgs"][hf]
            cur = S["cur"]

            def flat(t):
                return t[:, c0 : c0 + HSUB] if t[:].shape[1] == SUB else t[:]

            def grouped(t):
                return flat(t).rearrange("p (g s) -> p g s", g=HGRP)

            def slot(j):
                return _view(sqg[:], [(112, HGRP), (1, GRP)], GRP * j)

            ca, cb = cur[hf][a], cur[hf][b]
            a_final = last_touch[a] == (li, a, b)
            b_final = last_touch[b] == (li, a, b)
            if a_final:
                oa, ia, ib_a = slot(a), grouped(ca), grouped(cb)
            else:
                ta = o3pool.tile([HOR, HSUB], F32,
                                 name=f"s{st}_{hf}_{li}_{a}", tag="sorth",
                                 bufs=16)
                oa, ia, ib_a = ta[:], flat(ca), flat(cb)
            if b_final:
                ob, ia_b, ib_b = slot(b), grouped(ca), grouped(cb)
            else:
                tb = o3pool.tile([HOR, HSUB], F32,
                                 name=f"s{st}_{hf}_{li}_{b}", tag="sorth",
                                 bufs=16)
                ob, ia_b, ib_b = tb[:], flat(ca), flat(cb)
            nc.vector.tensor_tensor(oa, ia, ib_a, op=mybir.AluOpType.min)
            nc.vector.tensor_tensor(ob, ia_b, ib_b, op=mybir.AluOpType.max)
            if not a_final:
                cur[hf][a] = ta
            if not b_final:
                cur[hf][b] = tb

        def try_emit_ces():
            pending = S["pending"]
            o3 = S["o3"]
            progress = True
            while progress:
                progress = False
                for ce in list(pending):
                    li, a, b = ce
                    if o3[a] is None or o3[b] is None:
                        continue
                    blocked = any(
                        lj < li and (a in (a2, b2) or b in (a2, b2))
                        for (lj, a2, b2) in pending if (lj, a2, b2) != ce
                    )
                    if blocked:
                        continue
                    for hf in range(2):
                        emit_ce(hf, li, a, b)
                    pending.remove(ce)
                    progress = True

        def emit_weff(qh):
            h1T = [None] * 2
            for mc in range(2):
                ps = hpsum.tile([128, SUB], F32, tag="hps")
                for dc in range(4):
                    nc.tensor.matmul(
                        ps[:],
                        lhsT=w1_sb[qh][dc][:, ts(mc, 128)],
                        rhs=S["xT"][dc][:],
                        start=(dc == 0),
                        stop=(dc == 3),
                    )
                h1 = h1pool.tile([128, SUB], F32, name=f"h1_{st}_{qh}_{mc}",
                                  tag=f"h1_{qh % 2}_{mc}")
                nc.scalar.activation(
                    h1[:], ps[:], mybir.ActivationFunctionType.Relu,
                    bias=bias_sb[:, 4 + 2 * qh + mc : 5 + 2 * qh + mc],
                    scale=1.0,
                )
                h1T[mc] = h1
            return h1T

        def emit_w2(qh, h1T):
            ps = hpsum.tile([128, SUB], F32, tag="hps")
            for mc in range(2):
                nc.tensor.matmul(
                    ps[:], lhsT=w2_sb[qh][mc][:], rhs=h1T[mc][:],
                    start=(mc == 0), stop=(mc == 1),
                )
            h2 = h2pool.tile([128, SUB], F32, name=f"h2_{st}_{qh}",
                             tag=f"h2_{qh % 2}")
            nc.scalar.activation(
                h2[:], ps[:], mybir.ActivationFunctionType.Relu,
                bias=bias_sb[:, 18 + qh : 19 + qh], scale=1.0,
            )
            return h2

        def emit_w3(qh, h2):
            ps = hpsum.tile([HOR, SUB], F32, tag="hps")
            nc.tensor.matmul(
                ps[:], lhsT=w3_sb[qh][:, :], rhs=h2[:], start=True, stop=True
            )
            o = o3pool.tile([HOR, SUB], F32, name=f"o3_{st}_{qh}", tag="sortt",
                            bufs=9)
            nc.scalar.activation(
                o[:], ps[:], mybir.ActivationFunctionType.Identity,
                bias=bias_sb[:HOR, 25 + qh : 26 + qh], scale=1.0,
            )
            S["o3"][qh] = o
            S["cur"][0][qh] = o
            S["cur"][1][qh] = o

        def emit_pair_weff(k):
            if k == 0:
                # A tiles depend only on the aT DMA; building the first four
                # here lets DVE do this work during the head phase without
                # blocking the previous tile's compare-exchanges.
                for bi in range(4):
                    S["A"][bi] = build_A(bi)
            qa, qb = HEAD_PAIRS[k]
            h1a = emit_weff(qa)
            h1b = emit_weff(qb) if qb is not None else None
            S[("h1", k)] = (h1a, h1b)

        def emit_pair_tail(k):
            qa, qb = HEAD_PAIRS[k]
            h1a, h1b = S.pop(("h1", k))
            h2a = emit_w2(qa, h1a)
            h2b = emit_w2(qb, h1b) if qb is not None else None
            emit_w3(qa, h2a)
            if qb is not None:
                emit_w3(qb, h2b)
            try_emit_ces()
            if k == len(HEAD_PAIRS) - 1:
                assert not S["pending"]

        def emit_interp_batch(bi):
            ps_t = tpsum.tile([112, GB * HOR], F32, tag="tps")
            for j in range(GB):
                g = GB * bi + j
                hf, gl = divmod(g, HGRP)
                nc.tensor.transpose(
                    ps_t[:, ts(j, HOR)],
                    S["sqgs"][hf][:, 112 * gl : 112 * (gl + 1)],
                    ident[:HOR, :HOR],
                )
            sqa = sqTpool.tile([112, GB * HOR], F32, tag="sqa")
            nc.scalar.copy(sqa[:], ps_t[:])

            A = S["A"].pop(bi) if bi in S["A"] else build_A(bi)

            for j in range(GB):
                g = GB * bi + j
                gg = st * ngrp + g  # global group index
                rps = rpsum.tile([HOR, 512], F32, tag="rps")
                nc.tensor.matmul(
                    rps[:], lhsT=sqa[:, ts(j, HOR)],
                    rhs=A[:, ts(j, 512)], start=True, stop=True,
                )
                r_sb = rpool.tile([HOR, 512], BF16, tag="rsb")
                nc.scalar.copy(r_sb[:], rps[:])
                nc.sync.dma_start(
                    r_d[:, GRP * gg : GRP * (gg + 1), :],
                    r_sb[:].rearrange("p (s t) -> p s t", s=GRP),
                )

        return emit_loads, emit_pair_weff, emit_pair_tail, emit_interp_batch

    sts = [make_st(st) for st in range(n_sub)]
    n_pairs = len(HEAD_PAIRS)
    n_batches = ngrp // GB
    per_slot = -(-n_batches // n_pairs)  # interp batches woven per head pair
    sts[0][0]()  # loads for st 0
    for st in range(n_sub):
        if st + 1 < n_sub:
            sts[st + 1][0]()  # next tile's loads early
        # layer-level software pipeline: W_eff(k+1) is emitted before
        # W2/W3(k) so the per-pair serial evacuation tail never blocks the
        # next pair's independent matmuls in the in-order PE stream
        sts[st][1](0)
        for k in range(n_pairs):
            if k + 1 < n_pairs:
                sts[st][1](k + 1)
            sts[st][2](k)
            if st > 0:
                for bi in range(per_slot * k, min(per_slot * (k + 1),
                                                  n_batches)):
                    sts[st - 1][3](bi)
    for bi in range(n_batches):
        sts[n_sub - 1][3](bi)


# Per-instruction-type sync-wait slot capacity in the walrus ISA descriptors.
_WAIT_CAPACITY = {}  # default: every type gets a single wait slot
_DRAIN_CAPACITY = {
    "EngineType.SP": 1,
    "EngineType.PE": 1,
}


def _split_waits(nc):
    """Some walrus ISA descriptors (LDWEIGHTS, DMA) have too few sync-wait
    slots for the waits Tile emits.  Move surplus waits of overflowing
    instructions onto drains inserted right before them on the same queue."""
    for fn in nc.m.functions:
        for blk in fn.blocks:
            insts = list(blk.instructions)
            out = []
            changed = False
            for ins in insts:
                si = ins.sync_info
                cap = _WAIT_CAPACITY.get(type(ins).__name__, 1)
                if si is not None and si.on_wait and len(si.on_wait) > cap:
                    waits = list(si.on_wait)
                    surplus = waits[:-cap]
                    dcap = _DRAIN_CAPACITY.get(str(ins.engine), 1)
                    di = 0
                    while surplus:
                        chunk, surplus = surplus[:dcap], surplus[dcap:]
                        out.append(
                            mybir.InstDrain(
                                name=f"{ins.name}-wfence{di}",
                                engine=ins.engine,
                                ins=[],
                                outs=[],
                                sync_info=mybir.SyncInfo(
                                    on_wait=chunk, on_update=[]
                                ),
                            )
                        )
                        di += 1
                    si.on_wait = waits[-cap:]
                    changed = True
                out.append(ins)
            if changed:
                blk.instructions = out


def build_module(bc=BC):
    nc = bass.Bass("TRN2", target_bir_lowering=False, debug=False)
    xT_d = nc.dram_tensor("xT", [T, bc], F32, kind="ExternalInput").ap()
    aT_d = nc.dram_tensor("aT", [GRP * QF, (bc // GRP) * QT], F32,
                          kind="ExternalInput").ap()
    w1_d = nc.dram_tensor("Weff", [QF, T, H1], F32, kind="ExternalInput").ap()
    w2_d = nc.dram_tensor("W2", [QF, H1, H2], F32, kind="ExternalInput").ap()
    w3_d = nc.dram_tensor("W3", [QF, H2, HOR], F32, kind="ExternalInput").ap()
    bias_d = nc.dram_tensor("bias_all", [128, 32], F32, kind="ExternalInput").ap()
    m112_d = nc.dram_tensor("m112", [112, 512], F32, kind="ExternalInput").ap()
    r_d = nc.dram_tensor("r_out", [HOR, bc, QT], BF16, kind="ExternalOutput").ap()

    with tile.TileContext(nc) as tc:
        with ExitStack() as ctx:
            _emit(ctx, tc, (xT_d, aT_d, w1_d, w2_d, w3_d, bias_d, m112_d),
                  (r_d,), bc=bc)
    _split_waits(nc)
    return nc


_NC_CACHE = {}
LAST_RESULTS = None


def kernel(**inputs) -> np.ndarray:
    global LAST_RESULTS
    x = np.asarray(inputs["x"], dtype=np.float32)
    q = np.asarray(inputs["q"], dtype=np.float32)
    w_bb = np.asarray(inputs["W_bb"], dtype=np.float64)
    w1_64 = np.asarray(inputs["W1"], dtype=np.float64)
    # Fold the (linear, dropout-free) backbone into the first head layer:
    # h1 = relu(x @ (W_bb @ W1[q]) + (b_bb @ W1[q] + b1[q])).
    w_eff = np.ascontiguousarray(
        np.einsum("td,qdk->qtk", w_bb, w1_64).astype(np.float32)
    )
    b_eff = (
        np.asarray(inputs["b_bb"], dtype=np.float64) @ w1_64
        + np.asarray(inputs["b1"], dtype=np.float64)
    ).astype(np.float32)
    w2 = np.ascontiguousarray(np.asarray(inputs["W2"], dtype=np.float32))
    w3 = np.ascontiguousarray(np.asarray(inputs["W3"], dtype=np.float32))
    bias, m112 = _host_constants(
        b_eff,
        np.asarray(inputs["b2"], dtype=np.float32),
        np.asarray(inputs["b3"], dtype=np.float32),
    )
    q_sorted = np.sort(q, axis=1)

    if BC not in _NC_CACHE:
        _NC_CACHE[BC] = build_module(BC)
    nc = _NC_CACHE[BC]

    in_maps = []
    for c in range(NCORES):
        sl = slice(BC * c, BC * (c + 1))
        in_maps.append(
            {
                "xT": np.ascontiguousarray(x[sl].T),
                "aT": _host_coeffs(q_sorted[sl]),
                "Weff": w_eff,
                "W2": w2,
                "W3": w3,
                "bias_all": bias,
                "m112": m112,
            }
        )

    res = bass_utils.run_bass_kernel_spmd(nc, in_maps, core_ids=list(range(NCORES)))
    LAST_RESULTS = res
    out = np.empty((B, HOR, QT), dtype=np.float32)
    for c in range(NCORES):
        out[BC * c : BC * (c + 1)] = np.transpose(
            res.results[c]["r_out"].astype(np.float32), (1, 0, 2)
        )
    return out


# revision 36
# speedup vs baseline: 1.0094x; 1.0094x over previous
"""Trainium2 Bass kernel for nn_MultiHeadQuantileNBEATS.

Reference computation (per batch row b):
  feats = x @ W_bb + b_bb                                   [D]
  h1[q] = relu(feats @ W1[q] + b1[q])                       [QF, H1]
  h2[q] = relu(h1[q] @ W2[q] + b2[q])                       [QF, H2]
  o3[q] = h2[q] @ W3[q] + b3[q]                             [QF, HOR]
  sq    = sort(o3 over q)  (per (b, hor))                   [HOR, QF]
  out[b, h, t] = sort_t(interp(sq[b, h, :], q[b, t]))       [HOR, QT]

Device algorithm notes:
  * Pure data parallel over 8 cores (batch sharded, weights replicated).
  * All matmuls are true fp32 (the PE runs them in fused LOW_HIGH mode at
    ~2 cycles/row): the final interp is a convex combination whose result
    can be ~1e-3 while operands are ~0.5, so ANY reduced-precision
    arithmetic on the value path (fp32r/tf32 matmuls, bf16/fp16 storage of
    sorted values or coefficients) is amplified ~500x by the
    max(|expected|, 1e-3)-scaled error metric and fails the 2e-2 gate
    (measured: full fp32r scores 0.55).  Only the final output store is
    bf16 - rounding the result itself is a plain <=0.4% relative error.
  * The (linear, dropout-free) backbone is folded into the first head layer
    on the host: W_eff[q] = W_bb @ W1[q], b_eff[q] = b_bb @ W1[q] + b1[q] -
    this removes 25% of the PE work and the feats evacuations.
  * Head math is feature major ([feature, batch]); x arrives pre-transposed
    from the host so no on-chip input transpose is needed.
  * The final sort over QT is eliminated: the interpolant is monotone in the
    query level, so using host-sorted q yields already-sorted outputs.
  * Interpolation coefficients a_i(q[b,t]) are precomputed on the host in
    the transposed [(i,s), t] group layout (partition p = 16i+s keeps the
    device-side sorted-value writes contiguous), so the device only
    transposes the sorted head outputs (PE is_transpose path, 4 groups per
    PSUM bank with a single evacuation).
  * Engines execute their instruction streams in order, so EMISSION ORDER IS
    THE SCHEDULE.  The kernel is emitted as a software pipeline:
      - heads run in pairs with W_eff(pair k+1) emitted before W2/W3(pair k)
        so the serial psum->Act-evac->matmul chains of one pair hide behind
        the next pair's independent matmuls;
      - sort compare-exchanges (16-CE optimal network, DVE min/max on flat
        contiguous APs, two sample-halves) are emitted as soon as their head
        inputs exist, with head 0 computed last;
      - the previous super-tile's interp batches are woven between the next
        super-tile's head pairs so Act/DVE never build a stream backlog;
      - the block-diagonal interp coefficient matrix A (broadcast * 0/1
        mask, DVE) for the first 4 batches is built during the head phase.
  * Weights are DMA'd per head in consumption order; super-tile 0's inputs
    ride the Act HWDGE queue in parallel with weights on the sync queue.
  * Per-core output is written feature-major [HOR, B_core, QT] in bf16; the
    host upcasts and transposes to [B, HOR, QT] when gathering.
"""

import dataclasses
from contextlib import ExitStack

import numpy as np

import concourse.bass as bass
import concourse.mybir as mybir
import concourse.tile as tile
from concourse import bass_utils
from concourse.bass import ts
from concourse.masks import make_identity

F32 = mybir.dt.float32
BF16 = mybir.dt.bfloat16

B, T, D = 8192, 512, 512
H1, H2, HOR = 256, 128, 96
QF, QT = 7, 32
NCORES = 8
BC = B // NCORES  # batch per core
SUB = 512         # samples per super-tile
GRP = 16          # samples per interp group
GB = 4            # interp groups batched per PSUM bank / A tile
QUANTILE_LEVELS = np.array(
    [0.025, 0.1, 0.25, 0.5, 0.75, 0.9, 0.975], dtype=np.float32
)

# optimal 16-CE sorting network for 7 elements (ascending), disjoint layers
SORT7_LAYERS = [
    [(1, 2), (3, 4), (5, 6)],
    [(0, 2), (3, 5), (4, 6)],
    [(0, 1), (4, 5), (2, 6)],
    [(0, 4), (1, 5)],
    [(0, 3), (2, 5)],
    [(1, 3), (2, 4)],
    [(2, 3)],
]


def _view(ap, free_dims, extra_offset):
    """Rebuild an AP keeping its partition dim, with custom free-dim lattice."""
    dims = [tuple(ap.ap[0])] + [tuple(d) for d in free_dims]
    return dataclasses.replace(ap, ap=tuple(dims), offset=ap.offset + extra_offset)


# ---------------------------------------------------------------------------
# host-side constants / input prep
# ---------------------------------------------------------------------------

def _host_constants(b_eff, b2, b3):
    # bias_all [128, 32]: packed per-partition bias columns
    bias = np.zeros((128, 32), dtype=np.float32)
    for qh in range(QF):
        for mc in range(H1 // 128):
            bias[:, 4 + 2 * qh + mc] = b_eff[qh, 128 * mc : 128 * (mc + 1)]
        bias[:, 18 + qh] = b2[qh]
        bias[:96, 25 + qh] = b3[qh]
    # M112 [112, 512]: block-diagonal 0/1 mask over (sample, coeff) x (sample, t)
    m112 = np.zeros((112, 512), dtype=np.float32)
    for i in range(QF):
        for s in range(GRP):
            m112[GRP * i + s, QT * s : QT * s + QT] = 1.0
    return bias, m112


def _host_coeffs(q_sorted):
    """Interp coefficients for sorted q, in transposed group layout.

    Returns aT packed as [7*GRP, (B//GRP)*QT] fp32 where
    aT[7*sl + i, QT*g + t] = a_i(q_sorted[GRP*g + sl, t]).
    """
    ql = QUANTILE_LEVELS
    v = q_sorted  # [B, QT] fp32
    hi = np.clip(np.searchsorted(ql, v, side="left"), 1, QF - 1)
    lo = hi - 1
    w = ((v - ql[lo]) / (ql[hi] - ql[lo] + np.float32(1e-8))).astype(np.float32)
    a = np.zeros((v.shape[0], QT, QF), dtype=np.float32)
    np.put_along_axis(a, lo[:, :, None], (1.0 - w)[:, :, None], axis=2)
    np.put_along_axis(a, hi[:, :, None], w[:, :, None], axis=2)
    low_mask = v <= ql[0]
    high_mask = v >= ql[-1]
    a[low_mask] = 0.0
    a[high_mask] = 0.0
    a[..., 0] += low_mask.astype(np.float32)
    a[..., QF - 1] += high_mask.astype(np.float32)
    ngrp = v.shape[0] // GRP
    # [B, QT, QF] -> [ngrp, GRP(sl), QT, QF] -> [QF, GRP, ngrp, QT]
    # partition p = GRP*i + sl  (i-major keeps device-side writes contiguous)
    aT = a.reshape(ngrp, GRP, QT, QF).transpose(3, 1, 0, 2)
    return np.ascontiguousarray(aT.reshape(QF * GRP, ngrp * QT))


# ---------------------------------------------------------------------------
# device kernel
# ---------------------------------------------------------------------------

HEAD_ORDER = [1, 2, 3, 4, 5, 6, 0]
HEAD_PAIRS = [(1, 2), (3, 4), (5, 6), (0, None)]


def _emit(ctx: ExitStack, tc: tile.TileContext, ins, outs, bc=BC):
    nc = tc.nc
    xT_d, aT_d, w1_d, w2_d, w3_d, bias_d, m112_d = ins
    (r_d,) = outs
    n_sub = bc // SUB
    ngrp = SUB // GRP

    cpool = ctx.enter_context(tc.tile_pool(name="cpool", bufs=1))
    wpool = ctx.enter_context(tc.tile_pool(name="wpool", bufs=1))
    atpool = ctx.enter_context(tc.tile_pool(name="atpool", bufs=2))
    xTpool = ctx.enter_context(tc.tile_pool(name="xTpool", bufs=2))
    h1pool = ctx.enter_context(tc.tile_pool(name="h1pool", bufs=2))
    h2pool = ctx.enter_context(tc.tile_pool(name="h2pool", bufs=2))
    o3pool = ctx.enter_context(tc.tile_pool(name="o3pool", bufs=12))
    sqgpool = ctx.enter_context(tc.tile_pool(name="sqgpool", bufs=2))
    sqTpool = ctx.enter_context(tc.tile_pool(name="sqTpool", bufs=3))
    apool = ctx.enter_context(tc.tile_pool(name="apool", bufs=2))
    rpool = ctx.enter_context(tc.tile_pool(name="rpool", bufs=5))
    tpsum = ctx.enter_context(tc.tile_pool(name="tpsum", bufs=2, space="PSUM"))
    hpsum = ctx.enter_context(tc.tile_pool(name="hpsum", bufs=3, space="PSUM"))
    rpsum = ctx.enter_context(tc.tile_pool(name="rpsum", bufs=3, space="PSUM"))

    # --- constants ---
    ident = cpool.tile([128, 128], F32)
    make_identity(nc, ident[:])
    bias_sb = cpool.tile([128, 32], F32)
    nc.sync.dma_start(bias_sb[:], bias_d)
    m112 = cpool.tile([112, 512], F32)
    nc.sync.dma_start(m112[:], m112_d)

    # ---- super-tile 0 input loads FIRST so the PE can start as soon as the
    # first W_eff chunks land (weights queue behind on the same DMA queue) ----
    xT_st = []
    aT_st = []
    for st in range(n_sub):
        xT_st.append([None] * 4)
        aT_st.append(None)
    for tci in range(4):
        xt = xTpool.tile([128, SUB], F32, name=f"xT0_{tci}", tag=f"xT{tci}")
        nc.scalar.dma_start(xt[:], xT_d[ts(tci, 128), ts(0, SUB)])
        xT_st[0][tci] = xt

    # --- weights (persist across super-tiles); W_eff = W_bb @ W1 host-fused.
    # DMA'd per head in consumption order so the per-head pipelines never
    # wait on a weight that is queued behind unrelated ones ---
    w1_sb = [None] * QF
    w2_sb = [None] * QF
    w3_sb = [None] * QF
    for li_, qh in enumerate(HEAD_ORDER):
        row = []
        for dc in range(D // 128):
            w = wpool.tile([128, H1], F32, name=f"w1_{qh}_{dc}")
            nc.sync.dma_start(w[:], w1_d[qh, ts(dc, 128), :])
            row.append(w)
        w1_sb[qh] = row
        row = []
        for mc in range(H1 // 128):
            w = wpool.tile([128, H2], F32, name=f"w2_{qh}_{mc}")
            nc.sync.dma_start(w[:], w2_d[qh, ts(mc, 128), :])
            row.append(w)
        w2_sb[qh] = row
        w = wpool.tile([128, HOR], F32, name=f"w3_{qh}")
        nc.sync.dma_start(w[:], w3_d[qh])
        w3_sb[qh] = w
        if li_ == 1:
            aT0 = atpool.tile([112, ngrp * QT], F32, name="aT0", tag="aT")
            nc.scalar.dma_start(aT0[:], aT_d[:, ts(0, ngrp * QT)])
            aT_st[0] = aT0

    # =====================================================================
    # heads + sort + interpolation.  Work is emitted per super-tile, with the
    # previous super-tile's interp batches WOVEN between the next one's head
    # pairs: engines execute their streams in order, so emission order is the
    # schedule, and interleaving keeps Act/DVE from building a backlog that
    # stalls the PE's W2/W3 dependency chains.
    # =====================================================================
    HGRP = ngrp // 2   # groups per half
    HSUB = SUB // 2    # samples per half
    last_touch = {}
    for li, layer in enumerate(SORT7_LAYERS):
        for (a, b) in layer:
            last_touch[a] = (li, a, b)
            last_touch[b] = (li, a, b)

    def make_st(st):
        """Build the emission closures for one super-tile."""
        S = {}

        def emit_loads():
            if xT_st[st][0] is None:
                for tci in range(4):
                    xt = xTpool.tile([128, SUB], F32, name=f"xT{st}_{tci}",
                                     tag=f"xT{tci}")
                    nc.scalar.dma_start(xt[:], xT_d[ts(tci, 128), ts(st, SUB)])
                    xT_st[st][tci] = xt
                a_t = atpool.tile([112, ngrp * QT], F32, name=f"aT{st}",
                                  tag="aT")
                nc.scalar.dma_start(a_t[:], aT_d[:, ts(st, ngrp * QT)])
                aT_st[st] = a_t
            S["xT"] = xT_st[st]
            S["aT"] = aT_st[st]
            S["o3"] = [None] * QF
            S["cur"] = [{}, {}]
            S["pending"] = [(li, a, b) for li, layer in enumerate(SORT7_LAYERS)
                            for (a, b) in layer]
            S["sqgs"] = [
                sqgpool.tile([HOR, HGRP * 112], F32, name=f"sqg{st}_{hf}",
                             tag=f"sqg{hf}")
                for hf in range(2)
            ]
            S["A"] = {}

        def build_A(bi):
            A = apool.tile([112, GB * 512], F32, name=f"A{st}_{bi}", tag="A",
                           bufs=6)
            av = S["aT"][:, GB * QT * bi : GB * QT * (bi + 1)].rearrange(
                "p (j t) -> p j t", j=GB
            ).unsqueeze(2).broadcast_to((112, GB, GRP, QT))
            mv = m112[:].rearrange("p (s t) -> p s t", s=GRP).unsqueeze(
                1
            ).broadcast_to((112, GB, GRP, QT))
            Av = A[:].rearrange("p (j s t) -> p j s t", j=GB, s=GRP)
            nc.vector.tensor_tensor(Av, av, mv, op=mybir.AluOpType.mult)
            return A

        def emit_ce(hf, li, a, b):
            c0 = HSUB * hf
            sqg = S["sqgs"][hf]
            cur = S["cur"]

            def flat(t):
                return t[:, c0 : c0 + HSUB] if t[:].shape[1] == SUB else t[:]

            def grouped(t):
                return flat(t).rearrange("p (g s) -> p g s", g=HGRP)

            def slot(j):
                return _view(sqg[:], [(112, HGRP), (1, GRP)], GRP * j)

            ca, cb = cur[hf][a], cur[hf][b]
            a_final = last_touch[a] == (li, a, b)
            b_final = last_touch[b] == (li, a, b)
            if a_final:
                oa, ia, ib_a = slot(a), grouped(ca), grouped(cb)
            else:
                ta = o3pool.tile([HOR, HSUB], F32,
                                 name=f"s{st}_{hf}_{li}_{a}", tag="sorth",
                                 bufs=16)
                oa, ia, ib_a = ta[:], flat(ca), flat(cb)
            if b_final:
                ob, ia_b, ib_b = slot(b), grouped(ca), grouped(cb)
            else:
                tb = o3pool.tile([HOR, HSUB], F32,
                                 name=f"s{st}_{hf}_{li}_{b}", tag="sorth",
                                 bufs=16)
                ob, ia_b, ib_b = tb[:], flat(ca), flat(cb)
            nc.vector.tensor_tensor(oa, ia, ib_a, op=mybir.AluOpType.min)
            nc.vector.tensor_tensor(ob, ia_b, ib_b, op=mybir.AluOpType.max)
            if not a_final:
                cur[hf][a] = ta
            if not b_final:
                cur[hf][b] = tb

        def try_emit_ces():
            pending = S["pending"]
            o3 = S["o3"]
            progress = True
            while progress:
                progress = False
                for ce in list(pending):
                    li, a, b = ce
                    if o3[a] is None or o3[b] is None:
                        continue
                    blocked = any(
                        lj < li and (a in (a2, b2) or b in (a2, b2))
                        for (lj, a2, b2) in pending if (lj, a2, b2) != ce
                    )
                    if blocked:
                        continue
                    for hf in range(2):
                        emit_ce(hf, li, a, b)
                    pending.remove(ce)
                    progress = True

        def emit_weff(qh):
            h1T = [None] * 2
            for mc in range(2):
                ps = hpsum.tile([128, SUB], F32, tag="hps")
                for dc in range(4):
                    nc.tensor.matmul(
                        ps[:],
                        lhsT=w1_sb[qh][dc][:, ts(mc, 128)],
                        rhs=S["xT"][dc][:],
                        start=(dc == 0),
                        stop=(dc == 3),
                    )
                h1 = h1pool.tile([128, SUB], F32, name=f"h1_{st}_{qh}_{mc}",
                                  tag=f"h1_{qh % 2}_{mc}")
                nc.scalar.activation(
                    h1[:], ps[:], mybir.ActivationFunctionType.Relu,
                    bias=bias_sb[:, 4 + 2 * qh + mc : 5 + 2 * qh + mc],
                    scale=1.0,
                )
                h1T[mc] = h1
            return h1T

        def emit_w2(qh, h1T):
            ps = hpsum.tile([128, SUB], F32, tag="hps")
            for mc in range(2):
                nc.tensor.matmul(
                    ps[:], lhsT=w2_sb[qh][mc][:], rhs=h1T[mc][:],
                    start=(mc == 0), stop=(mc == 1),
                )
            h2 = h2pool.tile([128, SUB], F32, name=f"h2_{st}_{qh}",
                             tag=f"h2_{qh % 2}")
            nc.scalar.activation(
                h2[:], ps[:], mybir.ActivationFunctionType.Relu,
                bias=bias_sb[:, 18 + qh : 19 + qh], scale=1.0,
            )
            return h2

        def emit_w3(qh, h2):
            ps = hpsum.tile([HOR, SUB], F32, tag="hps")
            nc.tensor.matmul(
                ps[:], lhsT=w3_sb[qh][:, :], rhs=h2[:], start=True, stop=True
            )
            o = o3pool.tile([HOR, SUB], F32, name=f"o3_{st}_{qh}", tag="sortt",
                            bufs=9)
            nc.scalar.activation(
                o[:], ps[:], mybir.ActivationFunctionType.Identity,
                bias=bias_sb[:HOR, 25 + qh : 26 + qh], scale=1.0,
            )
            S["o3"][qh] = o
            S["cur"][0][qh] = o
            S["cur"][1][qh] = o

        def emit_pair_weff(k):
            if k == 0:
                # A tiles depend only on the aT DMA; building the first four
                # here lets DVE do this work during the head phase without
                # blocking the previous tile's compare-exchanges.
                for bi in range(4):
                    S["A"][bi] = build_A(bi)
            qa, qb = HEAD_PAIRS[k]
            h1a = emit_weff(qa)
            h1b = emit_weff(qb) if qb is not None else None
            S[("h1", k)] = (h1a, h1b)

        def emit_pair_tail(k):
            qa, qb = HEAD_PAIRS[k]
            h1a, h1b = S.pop(("h1", k))
            h2a = emit_w2(qa, h1a)
            h2b = emit_w2(qb, h1b) if qb is not None else None
            emit_w3(qa, h2a)
            if qb is not None:
                emit_w3(qb, h2b)
            try_emit_ces()
            if k == len(HEAD_PAIRS) - 1:
                assert not S["pending"]

        def emit_interp_batch(bi):
            ps_t = tpsum.tile([112, GB * HOR], F32, tag="tps")
            for j in range(GB):
                g = GB * bi + j
                hf, gl = divmod(g, HGRP)
                nc.tensor.transpose(
                    ps_t[:, ts(j, HOR)],
                    S["sqgs"][hf][:, 112 * gl : 112 * (gl + 1)],
                    ident[:HOR, :HOR],
                )
            sqa = sqTpool.tile([112, GB * HOR], F32, tag="sqa")
            nc.scalar.copy(sqa[:], ps_t[:])

            A = S["A"].pop(bi) if bi in S["A"] else build_A(bi)

            for j in range(GB):
                g = GB * bi + j
                gg = st * ngrp + g  # global group index
                rps = rpsum.tile([HOR, 512], F32, tag="rps")
                nc.tensor.matmul(
                    rps[:], lhsT=sqa[:, ts(j, HOR)],
                    rhs=A[:, ts(j, 512)], start=True, stop=True,
                )
                r_sb = rpool.tile([HOR, 512], BF16, tag="rsb")
                nc.scalar.copy(r_sb[:], rps[:])
                nc.sync.dma_start(
                    r_d[:, GRP * gg : GRP * (gg + 1), :],
                    r_sb[:].rearrange("p (s t) -> p s t", s=GRP),
                )

        return emit_loads, emit_pair_weff, emit_pair_tail, emit_interp_batch

    sts = [make_st(st) for st in range(n_sub)]
    n_pairs = len(HEAD_PAIRS)
    n_batches = ngrp // GB
    per_slot = -(-n_batches // n_pairs)  # interp batches woven per head pair
    sts[0][0]()  # loads for st 0
    for st in range(n_sub):
        if st + 1 < n_sub:
            sts[st + 1][0]()  # next tile's loads early
        # layer-level software pipeline: W_eff(k+1) is emitted before
        # W2/W3(k) so the per-pair serial evacuation tail never blocks the
        # next pair's independent matmuls in the in-order PE stream
        sts[st][1](0)
        for k in range(n_pairs):
            if k + 1 < n_pairs:
                sts[st][1](k + 1)
            sts[st][2](k)
            if st > 0:
                for bi in range(per_slot * k, min(per_slot * (k + 1),
                                                  n_batches)):
                    sts[st - 1][3](bi)
    for bi in range(n_batches):
        sts[n_sub - 1][3](bi)


# Per-instruction-type sync-wait slot capacity in the walrus ISA descriptors.
_WAIT_CAPACITY = {}  # default: every type gets a single wait slot
_DRAIN_CAPACITY = {
    "EngineType.SP": 1,
    "EngineType.PE": 1,
}


def _split_waits(nc):
    """Some walrus ISA descriptors (LDWEIGHTS, DMA) have too few sync-wait
    slots for the waits Tile emits.  Move surplus waits of overflowing
    instructions onto drains inserted right before them on the same queue."""
    for fn in nc.m.functions:
        for blk in fn.blocks:
            insts = list(blk.instructions)
            out = []
            changed = False
            for ins in insts:
                si = ins.sync_info
                cap = _WAIT_CAPACITY.get(type(ins).__name__, 1)
                if si is not None and si.on_wait and len(si.on_wait) > cap:
                    waits = list(si.on_wait)
                    surplus = waits[:-cap]
                    dcap = _DRAIN_CAPACITY.get(str(ins.engine), 1)
                    di = 0
                    while surplus:
                        chunk, surplus = surplus[:dcap], surplus[dcap:]
                        out.append(
                            mybir.InstDrain(
                                name=f"{ins.name}-wfence{di}",
                                engine=ins.engine,
                                ins=[],
                                outs=[],
                                sync_info=mybir.SyncInfo(
                                    on_wait=chunk, on_update=[]
                                ),
                            )
                        )
                        di += 1
                    si.on_wait = waits[-cap:]
                    changed = True
                out.append(ins)
            if changed:
                blk.instructions = out


def build_module(bc=BC):
    nc = bass.Bass("TRN2", target_bir_lowering=False, debug=False)
    xT_d = nc.dram_tensor("xT", [T, bc], F32, kind="ExternalInput").ap()
    aT_d = nc.dram_tensor("aT", [GRP * QF, (bc // GRP) * QT], F32,
                          kind="ExternalInput").ap()
    w1_d = nc.dram_tensor("Weff", [QF, T, H1], F32, kind="ExternalInput").ap()
    w2_d = nc.dram_tensor("W2", [QF, H1, H2], F32, kind="ExternalInput").ap()
    w3_d = nc.dram_tensor("W3", [QF, H2, HOR], F32, kind="ExternalInput").ap()
    bias_d = nc.dram_tensor("bias_all", [128, 32], F32, kind="ExternalInput").ap()
    m112_d = nc.dram_tensor("m112", [112, 512], F32, kind="ExternalInput").ap()
    r_d = nc.dram_tensor("r_out", [HOR, bc, QT], BF16, kind="ExternalOutput").ap()

    with tile.TileContext(nc) as tc:
        with ExitStack() as ctx:
            _emit(ctx, tc, (xT_d, aT_d, w1_d, w2_d, w3_d, bias_d, m112_d),
                  (r_d,), bc=bc)
    _split_waits(nc)
    return nc


_NC_CACHE = {}
LAST_RESULTS = None


def kernel(**inputs) -> np.ndarray:
    global LAST_RESULTS
    x = np.asarray(inputs["x"], dtype=np.float32)
    q = np.asarray(inputs["q"], dtype=np.float32)
    w_bb = np.asarray(inputs["W_bb"], dtype=np.float64)
    w1_64 = np.asarray(inputs["W1"], dtype=np.float64)
    # Fold the (linear, dropout-free) backbone into the first head layer:
    # h1 = relu(x @ (W_bb @ W1[q]) + (b_bb @ W1[q] + b1[q])).
    w_eff = np.ascontiguousarray(
        np.einsum("td,qdk->qtk", w_bb, w1_64).astype(np.float32)
    )
    b_eff = (
        np.asarray(inputs["b_bb"], dtype=np.float64) @ w1_64
        + np.asarray(inputs["b1"], dtype=np.float64)
    ).astype(np.float32)
    w2 = np.ascontiguousarray(np.asarray(inputs["W2"], dtype=np.float32))
    w3 = np.ascontiguousarray(np.asarray(inputs["W3"], dtype=np.float32))
    bias, m112 = _host_constants(
        b_eff,
        np.asarray(inputs["b2"], dtype=np.float32),
        np.asarray(inputs["b3"], dtype=np.float32),
    )
    q_sorted = np.sort(q, axis=1)

    if BC not in _NC_CACHE:
        _NC_CACHE[BC] = build_module(BC)
    nc = _NC_CACHE[BC]

    in_maps = []
    for c in range(NCORES):
        sl = slice(BC * c, BC * (c + 1))
        in_maps.append(
            {
                "xT": np.ascontiguousarray(x[sl].T),
                "aT": _host_coeffs(q_sorted[sl]),
                "Weff": w_eff,
                "W2": w2,
                "W3": w3,
                "bias_all": bias,
                "m112": m112,
            }
        )

    res = bass_utils.run_bass_kernel_spmd(nc, in_maps, core_ids=list(range(NCORES)))
    LAST_RESULTS = res
    out = np.empty((B, HOR, QT), dtype=np.float32)
    for c in range(NCORES):
        out[BC * c : BC * (c + 1)] = np.transpose(
            res.results[c]["r_out"].astype(np.float32), (1, 0, 2)
        )
    return out
